# revision 64
# baseline (speedup 1.0000x reference)
"""ArcFace loss kernel for 8 Trainium2 NeuronCores.

Strategy (class-parallel, Partial-FC style):
  - weight [100000, 512] is sharded along the class axis: 12500 classes per
    core (padded to 12544 = 7*1792), host-normalized and host-transposed.
  - Each core computes out[b, c] = <S*in_hat_b, w_hat_c> for its class range
    in natural [B, Cpad] layout and fp16; the host applies the ArcFace
    margin to the 512 (b, label[b]) cosines in float64 afterwards.
  - The matmul runs in fp8 DoubleRow mode (2 fp8 MACs per PE per cycle,
    2x the fp16 rate) with a 3-term residual decomposition that recovers
    bf16-level accuracy:
        X = S*x_hat;  W = w_hat
        x8 = e4m3(X), x8b = e4m3(X/32), rx = e5m2(X - x8)
        W8 = e4m3(32*W), R8 = e4m3(1024*(W - W8/32))
        psum = x8@W8 + x8b@R8 + rx@W8   (all at scale 32)
        out  = fp16(psum / 32)
    Quantization leftovers are each ~0.1-0.2% -> overall rel err ~1.6e-3.
  - Stationary = input (few LDWEIGHTS), PSUM groups of 448 classes across
    4 banks, copies split between VectorE and ScalarE with the 1/32 scale
    folded in, outputs in fp16.
  - Cold start: warm-up matmuls hold the PE p-state while the first DMAs
    land on the sync queue (the gpsimd/scalar DMA queues are ~4x slower);
    the s0 compute runs d-major over bt-pairs, W8 terms before R8 terms,
    so matmuls start as soon as the first weight half arrives.
"""

import math
import os
import sys

import numpy as np

for _p in ("/opt/trn_rl_repo",):
    if os.path.isdir(_p) and _p not in sys.path:
        sys.path.insert(0, _p)

import ml_dtypes

S = 30.0
MARGIN = 0.5
COS_M = math.cos(MARGIN)
SIN_M = math.sin(MARGIN)
TH = math.cos(math.pi - MARGIN)
MM = math.sin(math.pi - MARGIN) * MARGIN

B, D, C = 512, 512, 100000
NCORES = 8
CSH = C // NCORES            # 12500 classes per core
SUP = 7                      # weight "supers" per core
SUPC = 1792                  # classes per super
CPAD = SUP * SUPC            # 12544
GRP = 4                      # PSUM banks (class groups) per super
GN = SUPC // GRP             # 448 classes per group
BT = B // 128                # 4 batch tiles
DCH = D // 128               # 4 contraction chunks
DP = 2                       # DoubleRow d-pairs (K=256 each)
PSCALE = 32.0                # psum carries 32x the cosine
NWARM = 50                   # PE warm-up matmuls

# accumulation term order: W8 terms first so matmuls can start before the
# R8 halves arrive. (x-slot, w-slot): x8@W8, rx@W8, x8b@R8
TERMS = [(0, 0), (2, 0), (1, 1)]

LAST_RESULT = None
_CACHE = {}


def _build_nc():
    from concourse import bass, bacc, tile, mybir
    from contextlib import ExitStack

    f32 = mybir.dt.float32
    f16 = mybir.dt.float16
    e4 = mybir.dt.float8e4
    e5 = mybir.dt.float8e5
    DR = mybir.MatmulPerfMode.DoubleRowSwInterleave

    nc = bacc.Bacc()
    # x-side operands, host-interleaved in the DoubleRowSwInterleave
    # stationary format: per (dpair, bt) a [128, 256] slice holding A/B
    # k-tile pairs per PE column in reversed column order
    x4_e = nc.declare_dram_parameter("x4", [128, 2, DP, BT, 256], e4, isOutput=False)
    x5_e = nc.declare_dram_parameter("x5", [128, DP, BT, 256], e5, isOutput=False)
    # weights [dpair, w(W8/R8), s, p, t, col]: one DMA per (s, dpair, w)
    # moves 128 contiguous 3584B rows
    wt_e = nc.declare_dram_parameter(
        "wt", [DP, 2, SUP, 128, 2, SUPC], e4, isOutput=False
    )
    out_e = nc.declare_dram_parameter("out", [B, CPAD], f16, isOutput=True)

    with tile.TileContext(nc) as tc, ExitStack() as ctx:
        cpool = ctx.enter_context(tc.tile_pool(name="const", bufs=1))
        xpool = ctx.enter_context(tc.tile_pool(name="xin", bufs=1))
        wpool = ctx.enter_context(tc.tile_pool(name="wts", bufs=2))
        opool = ctx.enter_context(tc.tile_pool(name="outb", bufs=4))
        pm = ctx.enter_context(tc.tile_pool(name="pm", bufs=8, space="PSUM"))

        # warm-up constants (keep the PE busy while the first DMAs land);
        # memset on gpsimd, whose preamble retires earliest
        wstat = cpool.tile([128, 128], f16)
        nc.gpsimd.memset(wstat[:], 1.0)
        wmov = cpool.tile([128, 128], f16)
        nc.gpsimd.memset(wmov[:], 1.0)

        x4 = xpool.tile([128, 2, DP, BT, 256], e4)
        x5 = xpool.tile([128, DP, BT, 256], e5)

        warm = pm.tile([128, 512], f32, tag="pm")
        for i in range(NWARM):
            nc.tensor.matmul(
                warm[:, 0:128], wstat[:], wmov[:], start=True, stop=True
            )

        odmas = [nc.scalar.dma_start, nc.gpsimd.dma_start,
                 nc.sync.dma_start]

        def stat_ap(xslot, dp_, bt):
            if xslot == 2:
                return x5[:, dp_, bt, :]
            return x4[:, xslot, dp_, bt, :]

        for s in range(SUP):
            wts = {}
            for dp_ in range(DP):
                for wsl in range(2):
                    wt_t = wpool.tile(
                        [128, 2, SUPC], e4, tag=f"w{dp_}{wsl}",
                        name=f"w_{s}_{dp_}_{wsl}",
                    )
                    if s == 0 and dp_ == 0 and wsl == 0:
                        nc.sync.dma_start(x4[:], x4_e[:, :, :, :, :])
                    nc.sync.dma_start(wt_t[:], wt_e[dp_, wsl, s])
                    if s == 0 and dp_ == 0 and wsl == 0:
                        nc.sync.dma_start(x5[:], x5_e[:, :, :, :])
                    wts[(dp_, wsl)] = wt_t

            def emit_mms(pm_of, bts):
                # accumulate 6 DoubleRow matmuls per (bt, g): W8 terms for
                # both d-pairs first, then the R8 terms
                seq = [
                    (xslot, wslot, dp_)
                    for (xslot, wslot) in TERMS
                    for dp_ in range(DP)
                ]
                seq.sort(key=lambda t: t[1])  # W8 (wslot 0) before R8
                for i, (xslot, wslot, dp_) in enumerate(seq):
                    for g in range(GRP):
                        for bt in bts:
                            nc.tensor.matmul(
                                pm_of(bt, g)[:, 0:GN],
                                stat_ap(xslot, dp_, bt),
                                wts[(dp_, wslot)][:, :, g * GN:(g + 1) * GN],
                                start=(i == 0),
                                stop=(i == len(seq) - 1),
                                perf_mode=DR,
                            )

            if s == 0:
                # cold start: d-major over bt-pairs so every ready matmul
                # can issue while later weight chunks are still in flight
                for pair in ((0, 1), (2, 3)):
                    pms2 = {}
                    for bt in pair:
                        for g in range(GRP):
                            pms2[(bt, g)] = pm.tile(
                                [128, 512], f32, tag="pm", name=f"pm0_{bt}_{g}"
                            )
                    emit_mms(lambda bt, g: pms2[(bt, g)], pair)
                    for bt in pair:
                        ob = opool.tile([128, SUPC], f16, tag="ob")
                        for g in range(GRP):
                            dst = ob[:, g * GN:(g + 1) * GN]
                            src = pms2[(bt, g)][:, 0:GN]
                            if g % 2 == 0:
                                nc.vector.tensor_scalar_mul(dst, src, 1.0 / PSCALE)
                            else:
                                nc.scalar.mul(dst, src, 1.0 / PSCALE)
                        odmas[bt % 2](
                            out_e[bt * 128:(bt + 1) * 128, 0:SUPC],
                            ob[:],
                        )
                continue

            for bt in range(BT):
                pms = [
                    pm.tile([128, 512], f32, tag="pm", name=f"pm_{s}_{bt}_{g}")
                    for g in range(GRP)
                ]
                emit_mms(lambda bt_, g: pms[g], [bt])
                ob = opool.tile([128, SUPC], f16, tag="ob")
                # the very last output tiles ride the fast sync queue at
                # half-tile granularity; earlier tiles use the slow queues
                tail = s >= SUP - 2 and bt >= 2
                final = s == SUP - 1 and bt == BT - 1
                orows = out_e[bt * 128:(bt + 1) * 128, s * SUPC:(s + 1) * SUPC]
                for g in range(GRP):
                    dst = ob[:, g * GN:(g + 1) * GN]
                    src = pms[g][:, 0:GN]
                    if g % 2 == 0:
                        nc.vector.tensor_scalar_mul(dst, src, 1.0 / PSCALE)
                    else:
                        nc.scalar.mul(dst, src, 1.0 / PSCALE)
                    if final and g % 2 == 1:
                        nc.sync.dma_start(
                            orows[:, (g - 1) * GN:(g + 1) * GN],
                            ob[:, (g - 1) * GN:(g + 1) * GN],
                        )
                if final:
                    pass
                elif tail:
                    nc.sync.dma_start(orows, ob[:])
                elif s >= 3:
                    (nc.scalar.dma_start if (s * BT + bt) % 2 else nc.sync.dma_start)(
                        orows, ob[:]
                    )
                else:
                    odmas[(s * BT + bt) % 2](orows, ob[:])
    nc.finalize()
    return nc


def _get_nc():
    if "nc" not in _CACHE:
        _CACHE["nc"] = _build_nc()
    return _CACHE["nc"]


def _to_swi(a):
    """[b, d] -> [p, dpair, bt, 2u+t] with d = (dpair*2 + t)*128 + p and
    PE column m = 127 - u (DoubleRowSwInterleave stationary format)."""
    t1 = a.T.reshape(DP, 2, 128, BT, 128)[..., ::-1]   # [dpair, t, p, bt, u]
    t1 = t1.transpose(2, 0, 3, 4, 1)                    # [p, dpair, bt, u, t]
    return np.ascontiguousarray(t1.reshape(128, DP, BT, 256))


def kernel(input, label, weight):
    global LAST_RESULT
    from concourse.bass_utils import run_bass_kernel_spmd

    e4 = ml_dtypes.float8_e4m3
    e5 = ml_dtypes.float8_e5m2

    inp = np.asarray(input, dtype=np.float32)
    lbl = np.asarray(label).astype(np.int64)
    w = np.asarray(weight, dtype=np.float32)

    # host-side shard prep: normalize, fp8 residual split, shuffle
    xn = inp / np.maximum(np.linalg.norm(inp, axis=1, keepdims=True), 1e-12)
    X = (S * xn).astype(np.float32)
    x8 = X.astype(e4)
    x8b = (X / PSCALE).astype(e4)
    rx = (X - x8.astype(np.float32)).astype(e5)
    x4_host = np.ascontiguousarray(
        np.stack([_to_swi(x8), _to_swi(x8b)], axis=1)
    )  # [p, 2, dpair, bt, 256]
    x5_host = _to_swi(rx)  # [p, dpair, bt, 256]

    winv = 1.0 / np.maximum(np.linalg.norm(w, axis=1), 1e-12)
    wn = w * winv[:, None]
    wTf = np.zeros((NCORES, D, CPAD), dtype=np.float32)
    wTf[:, :, :CSH] = wn.reshape(NCORES, CSH, D).transpose(0, 2, 1)
    w8 = (PSCALE * wTf).astype(e4)
    # R8 = e4m3(1024 * (w_hat - W8/32)) = e4m3(32 * (32*w_hat - W8))
    r8 = (PSCALE * (PSCALE * wTf - w8.astype(np.float32))).astype(e4)
    wts = np.empty((NCORES, DP, 2, SUP, 128, 2, SUPC), dtype=e4)
    for kind, arr in ((0, w8), (1, r8)):
        # arr [k, D, CPAD]; d = (dpair*2 + t)*128 + p; c = s*SUPC + col
        a = arr.reshape(NCORES, DP, 2, 128, SUP, SUPC)
        wts[:, :, kind] = a.transpose(0, 1, 4, 3, 2, 5)  # [k, dpair, s, p, t, col]

    in_maps = [
        {"x4": x4_host, "x5": x5_host, "wt": np.ascontiguousarray(wts[k])}
        for k in range(NCORES)
    ]

    nc = _get_nc()
    res = run_bass_kernel_spmd(nc, in_maps, core_ids=list(range(NCORES)))
    LAST_RESULT = res
    outs = res.results

    full = np.empty((B, C), dtype=np.float32)
    for k in range(NCORES):
        blk = np.asarray(outs[k]["out"]).reshape(B, CPAD)[:, :CSH]
        full[:, k * CSH:(k + 1) * CSH] = blk.astype(np.float32)

    # apply the ArcFace margin to the 512 label positions (float64 on host)
    rows = np.arange(B)
    cosl = np.clip(full[rows, lbl].astype(np.float64) / S, -1.0, 1.0)
    sine = np.sqrt(np.clip(1.0 - cosl * cosl, 1e-9, 1.0))
    phi = cosl * COS_M - sine * SIN_M
    phi = np.where(cosl > TH, phi, cosl - MM)
    full[rows, lbl] = (S * phi).astype(np.float32)
    return full


# revision 65
# speedup vs baseline: 1.4232x; 1.4232x over previous
"""ArcFace loss kernel for 8 Trainium2 NeuronCores.

Strategy (class-parallel, Partial-FC style):
  - weight [100000, 512] is sharded along the class axis: 12500 classes per
    core (padded to 12544 = 7*1792). Shards are passed host-normalized,
    host-transposed ([D, Cpad]) in fp16 so the device streams them straight
    into the TensorEngine as the moving operand.
  - input [512, 512] is normalized and scaled by S on the host, transposed to
    [D, B] fp16, and broadcast to all cores as the stationary operand.
  - Each core computes out[b, c] = <S*in_hat_b, w_hat_c> for its class range
    in natural [B, Cpad] layout: per (super, b-tile) the 4 contraction chunks
    are accumulated into a 4-bank PSUM tile (4 groups of 448 classes), then
    evacuated to fp16 SBUF (VectorE + ScalarE split) and DMA'd out.
  - Stationary = input means only 4 LDWEIGHTS per (super, b-tile) instead of
    one per class chunk, and dummy warm-up matmuls keep the PE p-state
    ramping while the first weight DMAs land.
  - The ArcFace margin only affects one element per row (b, label[b]); the
    host applies the phi transform to those 512 gathered cosines in float64.
"""

import math
import os
import sys

import numpy as np

for _p in ("/opt/trn_rl_repo",):
    if os.path.isdir(_p) and _p not in sys.path:
        sys.path.insert(0, _p)

S = 30.0
MARGIN = 0.5
COS_M = math.cos(MARGIN)
SIN_M = math.sin(MARGIN)
TH = math.cos(math.pi - MARGIN)
MM = math.sin(math.pi - MARGIN) * MARGIN

B, D, C = 512, 512, 100000
NCORES = 8
CSH = C // NCORES            # 12500 classes per core
SUP = 7                      # weight "supers" per core
SUPC = 1792                  # classes per super
CPAD = SUP * SUPC            # 12544
GRP = 4                      # PSUM banks (class groups) per super
GN = SUPC // GRP             # 448 classes per group
BT = B // 128                # 4 batch tiles
DCH = D // 128               # 4 contraction chunks
NWARM = 50                   # PE warm-up matmuls

LAST_RESULT = None
_CACHE = {}


def _build_nc():
    from concourse import bass, bacc, tile, mybir
    from contextlib import ExitStack

    f32 = mybir.dt.float32
    f16 = mybir.dt.float16

    nc = bacc.Bacc()
    # stationary operand (S*input_hat).T, host-shuffled. xt0 carries the d0
    # chunk alone so the first matmuls gate on the fewest DMA packets; the
    # remaining d chunks ride one combined DMA (tile deps are whole-tile)
    xt0_e = nc.declare_dram_parameter("xt0", [128, 2, 256], f16, isOutput=False)
    xtr_e = nc.declare_dram_parameter(
        "xtr", [128, 2, DCH - 1, 256], f16, isOutput=False
    )
    wt_e = nc.declare_dram_parameter("wt", [D, CPAD], f16, isOutput=False)
    out_e = nc.declare_dram_parameter("out", [B, CPAD], f16, isOutput=True)

    with tile.TileContext(nc) as tc, ExitStack() as ctx:
        cpool = ctx.enter_context(tc.tile_pool(name="const", bufs=1))
        xpool = ctx.enter_context(tc.tile_pool(name="xin", bufs=1))
        wpool = ctx.enter_context(tc.tile_pool(name="wts", bufs=2))
        opool = ctx.enter_context(tc.tile_pool(name="outb", bufs=4))
        pm = ctx.enter_context(tc.tile_pool(name="pm", bufs=8, space="PSUM"))

        # warm-up constants (keep the PE busy while the first DMAs land);
        # memset on gpsimd, whose preamble retires earliest
        wstat = cpool.tile([128, 128], f16)
        nc.gpsimd.memset(wstat[:], 1.0)
        wmov = cpool.tile([128, 128], f16)
        nc.gpsimd.memset(wmov[:], 1.0)

        # stationary operand rides the fast sync queue (gpsimd/scalar DMA
        # queues are ~4x slower): xd0 first, then w_d0, then the rest
        in_d0 = xpool.tile([128, 2, 256], f16)
        in_dr = xpool.tile([128, 2, DCH - 1, 256], f16)

        warm = pm.tile([128, 512], f32, tag="pm")
        for i in range(NWARM):
            nc.tensor.matmul(
                warm[:, 0:128], wstat[:], wmov[:], start=True, stop=True
            )

        odmas = [nc.scalar.dma_start, nc.gpsimd.dma_start,
                 nc.sync.dma_start]

        def stat_ap(d, bt):
            h, j = divmod(bt, 2)
            if d == 0:
                return in_d0[:, h, j * 128:(j + 1) * 128]
            return in_dr[:, h, d - 1, j * 128:(j + 1) * 128]

        for s in range(SUP):
            wts = []
            for d in range(DCH):
                wt_t = wpool.tile([128, SUPC], f16, tag=f"w{d}")
                wsrc = wt_e[d * 128:(d + 1) * 128, s * SUPC:(s + 1) * SUPC]
                if s == 0 and d == 0:
                    nc.sync.dma_start(in_d0[:], xt0_e[:, :, :])
                nc.sync.dma_start(wt_t[:], wsrc)
                if s == 0 and d == 0:
                    nc.sync.dma_start(in_dr[:], xtr_e[:, :, :, :])
                wts.append(wt_t)

            if s == 0:
                # cold start: d-major, g-inner over bt-pairs so every ready
                # matmul can issue while later weight chunks are in flight
                for pair in ((0, 1), (2, 3)):
                    pms2 = {}
                    for bt in pair:
                        for g in range(GRP):
                            pms2[(bt, g)] = pm.tile(
                                [128, 512], f32, tag="pm", name=f"pm0_{bt}_{g}"
                            )
                    for d in range(DCH):
                        for g in range(GRP):
                            for bt in pair:
                                nc.tensor.matmul(
                                    pms2[(bt, g)][:, 0:GN],
                                    stat_ap(d, bt),
                                    wts[d][:, g * GN:(g + 1) * GN],
                                    start=(d == 0),
                                    stop=(d == DCH - 1),
                                )
                    for bt in pair:
                        ob = opool.tile([128, SUPC], f16, tag="ob")
                        for g in range(GRP):
                            eng = nc.vector.tensor_copy if g % 2 == 0 else nc.scalar.copy
                            eng(ob[:, g * GN:(g + 1) * GN], pms2[(bt, g)][:, 0:GN])
                        odmas[bt % 2](
                            out_e[bt * 128:(bt + 1) * 128, 0:SUPC],
                            ob[:],
                        )
                continue

            for bt in range(BT):
                pms = [
                    pm.tile([128, 512], f32, tag="pm", name=f"pm_{s}_{bt}_{g}")
                    for g in range(GRP)
                ]
                for d in range(DCH):
                    stat = stat_ap(d, bt)
                    for g in range(GRP):
                        nc.tensor.matmul(
                            pms[g][:, 0:GN],
                            stat,
                            wts[d][:, g * GN:(g + 1) * GN],
                            start=(d == 0),
                            stop=(d == DCH - 1),
                        )
                ob = opool.tile([128, SUPC], f16, tag="ob")
                # the very last output tiles ride the fast sync queue at
                # per-group granularity; earlier tiles use the slow queues
                tail = s >= SUP - 2 and bt >= 2
                final = s == SUP - 1 and bt == BT - 1
                orows = out_e[bt * 128:(bt + 1) * 128, s * SUPC:(s + 1) * SUPC]
                for g in range(GRP):
                    eng = nc.vector.tensor_copy if g % 2 == 0 else nc.scalar.copy
                    eng(ob[:, g * GN:(g + 1) * GN], pms[g][:, 0:GN])
                    if final and g % 2 == 1:
                        nc.sync.dma_start(
                            orows[:, (g - 1) * GN:(g + 1) * GN],
                            ob[:, (g - 1) * GN:(g + 1) * GN],
                        )
                if final:
                    pass
                elif tail:
                    nc.sync.dma_start(orows, ob[:])
                elif s >= 3:
                    # late outputs avoid the gpsimd queue: its teardown
                    # DRAIN cost sits on the critical path
                    (nc.scalar.dma_start if (s * BT + bt) % 2 else nc.sync.dma_start)(
                        orows, ob[:]
                    )
                else:
                    odmas[(s * BT + bt) % 2](orows, ob[:])
    nc.finalize()
    return nc


def _get_nc():
    if "nc" not in _CACHE:
        _CACHE["nc"] = _build_nc()
    return _CACHE["nc"]


def kernel(input, label, weight):
    global LAST_RESULT
    from concourse.bass_utils import run_bass_kernel_spmd

    inp = np.asarray(input, dtype=np.float32)
    lbl = np.asarray(label).astype(np.int64)
    w = np.asarray(weight, dtype=np.float32)

    # host-side shard prep: normalize, transpose, fp16-cast
    xn = inp / np.maximum(np.linalg.norm(inp, axis=1, keepdims=True), 1e-12)
    xs = (S * xn).T.astype(np.float16).reshape(DCH, 128, 2, 256)
    xs0 = np.ascontiguousarray(xs[0])            # [p, h, b%256]
    xsr = np.ascontiguousarray(xs[1:].transpose(1, 2, 0, 3))  # [p, h, d-1, b%256]

    winv = 1.0 / np.maximum(np.linalg.norm(w, axis=1), 1e-12)
    wn = w * winv[:, None]
    wT = np.zeros((NCORES, D, CPAD), dtype=np.float16)
    wT[:, :, :CSH] = wn.reshape(NCORES, CSH, D).transpose(0, 2, 1)

    in_maps = [
        {"xt0": xs0, "xtr": xsr, "wt": np.ascontiguousarray(wT[k])}
        for k in range(NCORES)
    ]

    nc = _get_nc()
    res = run_bass_kernel_spmd(nc, in_maps, core_ids=list(range(NCORES)))
    LAST_RESULT = res
    outs = res.results

    full = np.empty((B, C), dtype=np.float32)
    for k in range(NCORES):
        blk = np.asarray(outs[k]["out"]).reshape(B, CPAD)[:, :CSH]
        full[:, k * CSH:(k + 1) * CSH] = blk.astype(np.float32)

    # apply the ArcFace margin to the 512 label positions (float64 on host)
    rows = np.arange(B)
    cosl = np.clip(full[rows, lbl].astype(np.float64) / S, -1.0, 1.0)
    sine = np.sqrt(np.clip(1.0 - cosl * cosl, 1e-9, 1.0))
    phi = cosl * COS_M - sine * SIN_M
    phi = np.where(cosl > TH, phi, cosl - MM)
    full[rows, lbl] = (S * phi).astype(np.float32)
    return full


# revision 66
# speedup vs baseline: 1.4348x; 1.0081x over previous
"""ArcFace loss kernel for 8 Trainium2 NeuronCores.

Strategy (class-parallel, Partial-FC style):
  - weight [100000, 512] is sharded along the class axis: 12500 classes per
    core (padded to 12544 = 7*1792). Shards are passed host-normalized,
    host-transposed ([D, Cpad]) in fp16 so the device streams them straight
    into the TensorEngine as the moving operand.
  - input [512, 512] is normalized and scaled by S on the host, transposed
    and pre-shuffled to fp16, and broadcast to all cores as the stationary
    operand (only 4 LDWEIGHTS per super/b-tile instead of one per chunk).
  - Each core computes out[b, c] = <S*in_hat_b, w_hat_c> for its class range
    in natural [B, Cpad] layout: per (super, b-tile) the 4 contraction
    chunks accumulate into 4 single-bank PSUM groups of 448 classes, then
    are evacuated to fp16 SBUF (VectorE/ScalarE alternating) and DMA'd out.
  - Scheduling around measured bottlenecks: the PE needs 3us of continuous
    work to reach its 2.4GHz p-state (dummy warm-up matmuls bridge the
    ~12us DMA cold start); the sync-engine DMA queue is the only fast one
    (gpsimd/scalar queues are ~4x slower, used only for slack-rich middle
    outputs); tile dependencies are whole-tile, so operands are sized so
    each dependency is exactly one DMA; the final output tiles are split
    so the drain does not trail the last matmul.
  - The ArcFace margin only affects one element per row (b, label[b]); the
    host applies the phi transform to those 512 gathered cosines in float64.
"""

import math
import os
import sys

import numpy as np

for _p in ("/opt/trn_rl_repo",):
    if os.path.isdir(_p) and _p not in sys.path:
        sys.path.insert(0, _p)

S = 30.0
MARGIN = 0.5
COS_M = math.cos(MARGIN)
SIN_M = math.sin(MARGIN)
TH = math.cos(math.pi - MARGIN)
MM = math.sin(math.pi - MARGIN) * MARGIN

B, D, C = 512, 512, 100000
NCORES = 8
CSH = C // NCORES            # 12500 classes per core
SUP = 7                      # weight "supers" per core
SUPC = 1792                  # classes per super
CPAD = SUP * SUPC            # 12544
GRP = 4                      # PSUM banks (class groups) per super
GN = SUPC // GRP             # 448 classes per group
BT = B // 128                # 4 batch tiles
DCH = D // 128               # 4 contraction chunks
NWARM = 50                   # PE warm-up matmuls

LAST_RESULT = None
_CACHE = {}


def _build_nc():
    from concourse import bass, bacc, tile, mybir
    from contextlib import ExitStack

    f32 = mybir.dt.float32
    f16 = mybir.dt.float16

    nc = bacc.Bacc()
    # stationary operand (S*input_hat).T, host-shuffled. xt0 carries the d0
    # chunk alone so the first matmuls gate on the fewest DMA packets; the
    # remaining d chunks ride one combined DMA (tile deps are whole-tile)
    xt0_e = nc.declare_dram_parameter("xt0", [128, 2, 256], f16, isOutput=False)
    xtr_e = nc.declare_dram_parameter(
        "xtr", [128, 2, DCH - 1, 256], f16, isOutput=False
    )
    wt_e = nc.declare_dram_parameter("wt", [D, CPAD], f16, isOutput=False)
    out_e = nc.declare_dram_parameter("out", [B, CPAD], f16, isOutput=True)

    with tile.TileContext(nc) as tc, ExitStack() as ctx:
        cpool = ctx.enter_context(tc.tile_pool(name="const", bufs=1))
        xpool = ctx.enter_context(tc.tile_pool(name="xin", bufs=1))
        wpool = ctx.enter_context(tc.tile_pool(name="wts", bufs=2))
        opool = ctx.enter_context(tc.tile_pool(name="outb", bufs=4))
        pm = ctx.enter_context(tc.tile_pool(name="pm", bufs=8, space="PSUM"))

        # warm-up constants (keep the PE busy while the first DMAs land);
        # memset on gpsimd, whose preamble retires earliest
        wstat = cpool.tile([128, 128], f16)
        nc.gpsimd.memset(wstat[:], 1.0)
        wmov = cpool.tile([128, 128], f16)
        nc.gpsimd.memset(wmov[:], 1.0)

        # stationary operand rides the fast sync queue (gpsimd/scalar DMA
        # queues are ~4x slower): xd0 first, then w_d0, then the rest
        in_d0 = xpool.tile([128, 2, 256], f16)
        in_dr = xpool.tile([128, 2, DCH - 1, 256], f16)

        warm = pm.tile([128, 512], f32, tag="pm")
        for i in range(NWARM):
            nc.tensor.matmul(
                warm[:, 0:128], wstat[:], wmov[:], start=True, stop=True
            )

        odmas = [nc.scalar.dma_start, nc.gpsimd.dma_start,
                 nc.sync.dma_start]

        def stat_ap(d, bt):
            h, j = divmod(bt, 2)
            if d == 0:
                return in_d0[:, h, j * 128:(j + 1) * 128]
            return in_dr[:, h, d - 1, j * 128:(j + 1) * 128]

        for s in range(SUP):
            wts = []
            for d in range(DCH):
                wt_t = wpool.tile([128, SUPC], f16, tag=f"w{d}")
                wsrc = wt_e[d * 128:(d + 1) * 128, s * SUPC:(s + 1) * SUPC]
                if s == 0 and d == 0:
                    nc.sync.dma_start(in_d0[:], xt0_e[:, :, :])
                nc.sync.dma_start(wt_t[:], wsrc)
                if s == 0 and d == 0:
                    nc.sync.dma_start(in_dr[:], xtr_e[:, :, :, :])
                wts.append(wt_t)

            if s == 0:
                # cold start: d-major, g-inner over bt-pairs so every ready
                # matmul can issue while later weight chunks are in flight
                for pair in ((0, 1), (2, 3)):
                    pms2 = {}
                    for bt in pair:
                        for g in range(GRP):
                            pms2[(bt, g)] = pm.tile(
                                [128, 512], f32, tag="pm", name=f"pm0_{bt}_{g}"
                            )
                    for d in range(DCH):
                        for g in range(GRP):
                            for bt in pair:
                                nc.tensor.matmul(
                                    pms2[(bt, g)][:, 0:GN],
                                    stat_ap(d, bt),
                                    wts[d][:, g * GN:(g + 1) * GN],
                                    start=(d == 0),
                                    stop=(d == DCH - 1),
                                )
                    for bt in pair:
                        ob = opool.tile([128, SUPC], f16, tag="ob")
                        for g in range(GRP):
                            eng = nc.vector.tensor_copy if g % 2 == 0 else nc.scalar.copy
                            eng(ob[:, g * GN:(g + 1) * GN], pms2[(bt, g)][:, 0:GN])
                        odmas[bt % 2](
                            out_e[bt * 128:(bt + 1) * 128, 0:SUPC],
                            ob[:],
                        )
                continue

            for bt in range(BT):
                pms = [
                    pm.tile([128, 512], f32, tag="pm", name=f"pm_{s}_{bt}_{g}")
                    for g in range(GRP)
                ]
                for d in range(DCH):
                    stat = stat_ap(d, bt)
                    for g in range(GRP):
                        nc.tensor.matmul(
                            pms[g][:, 0:GN],
                            stat,
                            wts[d][:, g * GN:(g + 1) * GN],
                            start=(d == 0),
                            stop=(d == DCH - 1),
                        )
                ob = opool.tile([128, SUPC], f16, tag="ob")
                # the very last output tiles ride the fast sync queue at
                # per-group granularity; earlier tiles use the slow queues
                tail = s >= SUP - 2 and bt >= 2
                final = s == SUP - 1 and bt == BT - 1
                orows = out_e[bt * 128:(bt + 1) * 128, s * SUPC:(s + 1) * SUPC]
                for g in range(GRP):
                    eng = nc.vector.tensor_copy if g % 2 == 0 else nc.scalar.copy
                    eng(ob[:, g * GN:(g + 1) * GN], pms[g][:, 0:GN])
                    if final and g % 2 == 1:
                        nc.sync.dma_start(
                            orows[:, (g - 1) * GN:(g + 1) * GN],
                            ob[:, (g - 1) * GN:(g + 1) * GN],
                        )
                if final:
                    pass
                elif tail:
                    nc.sync.dma_start(orows, ob[:])
                elif s >= 3:
                    # late outputs avoid the gpsimd queue: its teardown
                    # DRAIN cost sits on the critical path
                    (nc.scalar.dma_start if (s * BT + bt) % 2 else nc.sync.dma_start)(
                        orows, ob[:]
                    )
                else:
                    odmas[(s * BT + bt) % 2](orows, ob[:])
    nc.finalize()
    return nc


def _get_nc():
    if "nc" not in _CACHE:
        _CACHE["nc"] = _build_nc()
    return _CACHE["nc"]


def kernel(input, label, weight):
    global LAST_RESULT
    from concourse.bass_utils import run_bass_kernel_spmd

    inp = np.asarray(input, dtype=np.float32)
    lbl = np.asarray(label).astype(np.int64)
    w = np.asarray(weight, dtype=np.float32)

    # host-side shard prep: normalize, transpose, fp16-cast
    xn = inp / np.maximum(np.linalg.norm(inp, axis=1, keepdims=True), 1e-12)
    xs = (S * xn).T.astype(np.float16).reshape(DCH, 128, 2, 256)
    xs0 = np.ascontiguousarray(xs[0])            # [p, h, b%256]
    xsr = np.ascontiguousarray(xs[1:].transpose(1, 2, 0, 3))  # [p, h, d-1, b%256]

    winv = 1.0 / np.maximum(np.linalg.norm(w, axis=1), 1e-12)
    wn = w * winv[:, None]
    wT = np.zeros((NCORES, D, CPAD), dtype=np.float16)
    wT[:, :, :CSH] = wn.reshape(NCORES, CSH, D).transpose(0, 2, 1)

    in_maps = [
        {"xt0": xs0, "xtr": xsr, "wt": np.ascontiguousarray(wT[k])}
        for k in range(NCORES)
    ]

    nc = _get_nc()
    res = run_bass_kernel_spmd(nc, in_maps, core_ids=list(range(NCORES)))
    LAST_RESULT = res
    outs = res.results

    full = np.empty((B, C), dtype=np.float32)
    for k in range(NCORES):
        blk = np.asarray(outs[k]["out"]).reshape(B, CPAD)[:, :CSH]
        full[:, k * CSH:(k + 1) * CSH] = blk.astype(np.float32)

    # apply the ArcFace margin to the 512 label positions (float64 on host)
    rows = np.arange(B)
    cosl = np.clip(full[rows, lbl].astype(np.float64) / S, -1.0, 1.0)
    sine = np.sqrt(np.clip(1.0 - cosl * cosl, 1e-9, 1.0))
    phi = cosl * COS_M - sine * SIN_M
    phi = np.where(cosl > TH, phi, cosl - MM)
    full[rows, lbl] = (S * phi).astype(np.float32)
    return full
